# revision 59
# baseline (speedup 1.0000x reference)
"""Trainium2 Bass kernel for nn_MaxGraphConv (gnn_message_passing).

Reference computation (per batch element, all f32):
  xn   = L2-normalize(x^T along C)                       # (N, C)
  d2   = |xn_i - xn_j|^2 via Gram matrix, self excluded
  idx  = 16 nearest neighbors per point (smallest d2)
  md_c = max_k |xn_ic - xn_jc| over the 16 neighbors      # (N, C)
  feat = interleave(xn, md) -> (2C, N); y = W @ feat + b
  y    = BatchNorm(training stats over (B, N)) ; out = gelu_exact(y)

Sharding: data-parallel over B across 8 cores (2 batches/core); conv/BN
params replicated; BN statistics all-reduced (4KB) on device.

Device algorithm per batch (fp16 compute, f32 norms/scores/stats):
  * Since xn is L2-normalized, |xn_m|^2 == 1, so ranking by distance is
    ranking by the raw Gram G = xn^T xn DESCENDING. No score correction.
  * Gram via PE fp16 matmuls into a 2-bank PSUM tile; the diagonal is
    suppressed by accumulating -60000*I with one extra PE matmul.
  * top-16 via DVE InstMax/InstMaxIndex/InstMatchReplace (8 + 8) run
    directly on the f32 PSUM scores (no SBUF copy, exact selection).
  * (N, C)-layout xn via DMA-transpose XBAR (16-bit), written once to
    DRAM as the gather source; the 16 neighbor rows per point fetched
    with one indirect DMA per neighbor slot ([128,1] offsets; the
    first 8 issue as soon as the first max_index lands).
  * md from min/max TT trees over the 16 gathered rows (DVE, fp16 2x);
    md = max(xn - min, max - xn).
  * conv as W_even @ xn + W_odd @ md (W pre-split+transposed on host)
    in per-(out-tile, half) pieces interleaved into the next batch's
    row-block loop; the row-block loop itself is 3-stage
    software-pipelined so the in-order DVE never waits on gathers.
  * BN: per-channel sum/sumsq (ACT accum during PSUM->SBUF copy and a
    Square pass) -> 4KB AllReduce -> affine+gelu fused on ACT.
  * conv bias b cancels exactly in training-mode BN (y+b shifts the mean
    by b) so it is accepted and ignored.
"""

import sys

if "/opt/trn_rl_repo" not in sys.path:
    sys.path.insert(0, "/opt/trn_rl_repo")

import numpy as np

import concourse.bacc as bacc
import concourse.mybir as mybir
import concourse.tile as tile
from concourse import bass
from concourse.alu_op_type import AluOpType
from concourse.bass import IndirectOffsetOnAxis
from concourse.bass_utils import run_bass_kernel_spmd
from concourse.masks import make_identity

F32 = mybir.dt.float32
F16 = mybir.dt.float16
U32 = mybir.dt.uint32
U16 = mybir.dt.uint16
I16 = mybir.dt.int16
AF = mybir.ActivationFunctionType
AX = None  # set lazily (bass_rust.AxisListType.X)

N_CORES = 8
B, C, N = 16, 256, 1024
B_LOC = B // N_CORES          # 2 batches per core
OUT = 2 * C                   # 512
K_G = 16
BN_EPS = 1e-5
BIG = 1.0e30                  # f32 "infinity" for masking PSUM scores
NB = N // 128                 # 8 row blocks per batch
CT = C // 128                 # 2 channel tiles
OT = OUT // 128               # 4 out-channel tiles
NH = N // 512                 # 2 free-dim halves for matmul


def build_kernel(use_gelu=True, collective=True, dbg=False):
    import bass_rust

    global AX
    AX = bass_rust.AxisListType.X

    nc = bacc.Bacc("TRN2", target_bir_lowering=False, debug=False)

    x_in = nc.dram_tensor("x", [B_LOC, C, N], F32, kind="ExternalInput")
    wev_in = nc.dram_tensor("wev", [C, OUT], F16, kind="ExternalInput")
    wod_in = nc.dram_tensor("wod", [C, OUT], F16, kind="ExternalInput")
    gamma_in = nc.dram_tensor("gamma4", [128, OT], F32, kind="ExternalInput")
    beta_in = nc.dram_tensor("beta4", [128, OT], F32, kind="ExternalInput")
    out_dram = nc.dram_tensor("out", [B_LOC, OUT, N], F16, kind="ExternalOutput")

    # gather sources (offset-0 requirement for indirect DMA src)
    xn_rows = [nc.dram_tensor(f"xn_rows{bi}", [N, C], F16) for bi in range(B_LOC)]
    stats_in = nc.dram_tensor("stats_in", [128, 2 * OT], F32)
    stats_out = nc.dram_tensor("stats_out", [128, 2 * OT], F32)
    if dbg:
        idx_dump = nc.dram_tensor(
            "idx_dump", [B_LOC, NB, 128, K_G], U32, kind="ExternalOutput"
        )
        md_dump = nc.dram_tensor(
            "md_dump", [B_LOC, CT * 128, N], F16, kind="ExternalOutput"
        )
        nbr_dump = nc.dram_tensor(
            "nbr_dump", [B_LOC, NB, 128, K_G, C], F16, kind="ExternalOutput"
        )
        y_dump = nc.dram_tensor(
            "y_dump", [B_LOC, OUT, N], F16, kind="ExternalOutput"
        )
        s_dump = nc.dram_tensor(
            "s_dump", [128, 4 * OT], F32, kind="ExternalOutput"
        )

    from contextlib import ExitStack

    with tile.TileContext(nc) as tc, ExitStack() as ctx:
        ep = ctx.enter_context
        constp = ep(tc.tile_pool(name="const", bufs=1))
        wpool = ep(tc.tile_pool(name="wpool", bufs=CT))
        xload = ep(tc.tile_pool(name="xload", bufs=2))
        sqp = ep(tc.tile_pool(name="sqp", bufs=2))
        rowp = ep(tc.tile_pool(name="rowp", bufs=2))
        bcp = ep(tc.tile_pool(name="bcp", bufs=2))
        xnp = ep(tc.tile_pool(name="xnp", bufs=2 * CT))
        xnncp = ep(tc.tile_pool(name="xnnc", bufs=2))
        idxp = ep(tc.tile_pool(name="idxp", bufs=6))
        nbrp = ep(tc.tile_pool(name="nbrp", bufs=4))
        treep = ep(tc.tile_pool(name="treep", bufs=2))
        mdncp = ep(tc.tile_pool(name="mdnc", bufs=4))
        mdcnp = ep(tc.tile_pool(name="mdcn", bufs=CT))
        ypool = ep(tc.tile_pool(name="ypool", bufs=2 * OT))
        outp = ep(tc.tile_pool(name="outp", bufs=2))
        statp = ep(tc.tile_pool(name="statp", bufs=1))
        ps_tp = ep(tc.tile_pool(name="ps_tp", bufs=2, space="PSUM"))
        ps_mm = ep(tc.tile_pool(name="ps_mm", bufs=2, space="PSUM"))

        # ---- constants ----
        ident_h = constp.tile([128, 128], F16)
        make_identity(nc, ident_h[:])
        ones_col = constp.tile([128, 1], F32)
        nc.vector.memset(ones_col[:], 1.0)

        # per-channel partial sums of y and y^2: col = (ot*2 + bi)*2 + h
        part_s1 = statp.tile([128, OT * B_LOC * NH], F32)
        part_s2 = statp.tile([128, OT * B_LOC * NH], F32)

        y_tiles = {}  # (bi, ot) -> tile (128, N) fp16

        # ---- phase 1 (both batches): load, norms, normalize, layouts ----
        xn_ct_b = {}
        xn_nc_b = {}
        for bi in range(B_LOC):
            x_ct = [
                xload.tile([128, N], F32, tag="x", name=f"x{bi}_{ct}")
                for ct in range(CT)
            ]
            xsq_ct = [
                sqp.tile([128, N], F32, tag="xsq", name=f"xsq{bi}_{ct}")
                for ct in range(CT)
            ]
            # chunk loads and squares by half so the norm chain starts early
            for h in range(NH):
                hs = slice(h * 512, (h + 1) * 512)
                for ct in range(CT):
                    nc.sync.dma_start(
                        out=x_ct[ct][:, hs],
                        in_=x_in[bi, ct * 128:(ct + 1) * 128, hs],
                    )
                    nc.scalar.activation(
                        xsq_ct[ct][:, hs], x_ct[ct][:, hs], AF.Square
                    )

            rnorm_row = rowp.tile([1, N], F32, tag="rnorm")
            for h in range(NH):
                hs = slice(h * 512, (h + 1) * 512)
                ps = ps_mm.tile([1, 512], F32, tag="cv", name=f"srow{bi}_{h}")
                for ct in range(CT):
                    nc.tensor.matmul(
                        out=ps[:],
                        lhsT=ones_col[:],
                        rhs=xsq_ct[ct][:, hs],
                        start=(ct == 0),
                        stop=(ct == CT - 1),
                    )
                # sqrt on ACT (PSUM -> SBUF), then reciprocal on DVE
                srt = rowp.tile([1, 512], F32, tag="srt")
                nc.scalar.activation(srt[:], ps[:], AF.Sqrt)
                nc.vector.reciprocal(rnorm_row[:, hs], srt[:])

            # broadcast row across partitions (GpSimd custom op)
            rnorm_bc = bcp.tile([128, N], F32, tag="rnorm_bc")
            nc.gpsimd.partition_broadcast(rnorm_bc[:], rnorm_row[:])

            # normalize: xn = x * rnorm (column-wise), fp16 out
            xn_ct = []
            for ct in range(CT):
                t = xnp.tile([128, N], F16, tag="xn")
                nc.vector.tensor_tensor(
                    t[:], x_ct[ct][:], rnorm_bc[:], op=AluOpType.mult
                )
                xn_ct.append(t)
            xn_ct_b[bi] = xn_ct

            # (N, C) layout via DMA-transpose XBAR; gather rows to DRAM
            xn_nc = xnncp.tile([128, NB, C], F16, tag="xn_nc")
            for ct in range(CT):
                nc.sync.dma_start(
                    out=xn_nc[:, :, ct * 128:(ct + 1) * 128],
                    in_=xn_ct[ct][:],
                    transpose=True,
                )
            nc.sync.dma_start(
                out=xn_rows[bi][:, :].rearrange("(rb p) c -> p rb c", p=128),
                in_=xn_nc[:],
            )
            xn_nc_b[bi] = xn_nc

        # ---- replicated weights / BN params (needed late; after x loads
        # so batch 0's norm chain leads the DMA queue) ----
        nbig_h = constp.tile([128, 128], F16)
        nc.scalar.activation(nbig_h[:], ident_h[:], AF.Copy, scale=-60000.0)
        wev = []
        wod = []
        for ct in range(CT):
            t = wpool.tile([128, OUT], F16, tag="wev", name=f"wev{ct}")
            nc.sync.dma_start(out=t[:], in_=wev_in[ct * 128:(ct + 1) * 128, :])
            wev.append(t)
            t = wpool.tile([128, OUT], F16, tag="wod", name=f"wod{ct}")
            nc.sync.dma_start(out=t[:], in_=wod_in[ct * 128:(ct + 1) * 128, :])
            wod.append(t)
        gamma4 = constp.tile([128, OT], F32)
        nc.sync.dma_start(out=gamma4[:], in_=gamma_in[:, :])
        beta4 = constp.tile([128, OT], F32)
        nc.sync.dma_start(out=beta4[:], in_=beta_in[:, :])

        # ---- per batch: row blocks (software-pipelined) then conv ----
        for bi in range(B_LOC):
            xn_ct = xn_ct_b[bi]
            xn_nc = xn_nc_b[bi]

            if bi == 0:
                pending_conv = []

            md_cn = []
            for ct in range(CT):
                md_cn.append(
                    mdcnp.tile([128, N], F16, tag="md_cn", name=f"md_cn{bi}_{ct}")
                )

            nbr_t = {}
            def conv_piece(ot, h, bi=bi, xn_ct=xn_ct, md_cn=md_cn):
                ots = slice(ot * 128, (ot + 1) * 128)
                if h == 0:
                    yt = ypool.tile(
                        [128, N], F16, tag="y", name=f"y{bi}_{ot}"
                    )
                    y_tiles[(bi, ot)] = yt
                else:
                    yt = y_tiles[(bi, ot)]
                hs = slice(h * 512, (h + 1) * 512)
                ps = ps_mm.tile(
                    [128, 512], F32, tag="cv", name=f"cps{bi}_{ot}_{h}"
                )
                for ct in range(CT):
                    nc.tensor.matmul(
                        out=ps[:],
                        lhsT=wev[ct][:, ots],
                        rhs=xn_ct[ct][:, hs],
                        start=(ct == 0),
                        stop=False,
                    )
                for ct in range(CT):
                    nc.tensor.matmul(
                        out=ps[:],
                        lhsT=wod[ct][:, ots],
                        rhs=md_cn[ct][:, hs],
                        start=False,
                        stop=(ct == CT - 1),
                    )
                # move PSUM->SBUF on ACT with fused per-channel sum
                col = (ot * B_LOC + bi) * NH + h
                nc.scalar.activation(
                    yt[:, hs],
                    ps[:],
                    AF.Copy,
                    accum_out=part_s1[:, col:col + 1],
                )
                # sumsq via ACT Square with fused per-channel sum
                sq_scr = sqp.tile(
                    [128, 512], F16, tag="ysq", name=f"ysq{bi}_{ot}_{h}"
                )
                nc.scalar.activation(
                    sq_scr[:],
                    yt[:, hs],
                    AF.Square,
                    accum_out=part_s2[:, col:col + 1],
                )
                if dbg and h == NH - 1:
                    nc.sync.dma_start(
                        out=y_dump[bi, ot * 128:(ot + 1) * 128, :], in_=yt[:]
                    )


            my_h0_pieces = [
                (lambda ot=ot, f=conv_piece: f(ot, 0)) for ot in range(OT)
            ]
            my_h1_pieces = [
                (lambda ot=ot, f=conv_piece: f(ot, 1)) for ot in range(OT)
            ]


            def stage_a(rb, bi=bi, xn_ct=xn_ct):
                rbs = slice(rb * 128, (rb + 1) * 128)
                # Gram row block straight into a 2-bank PSUM tile; the top-k
                # scans read PSUM directly (f32 scores, no SBUF copy).
                ps = ps_mm.tile([128, N], F32, tag="mm")
                for h in range(NH):
                    hs = slice(h * 512, (h + 1) * 512)
                    # matmul group per half: 2 channel tiles, plus (for the
                    # half containing the diagonal) a -60000*I accumulation
                    # for self-exclusion (frees a DVE op per row block)
                    group = [
                        (ps[:, hs], xn_ct[ct][:, rbs], xn_ct[ct][:, hs])
                        for ct in range(CT)
                    ]
                    if h == rb // (NB // NH):
                        group.append(
                            (
                                ps[:, rb * 128:(rb + 1) * 128],
                                nbig_h[:],
                                ident_h[:],
                            )
                        )
                    for k, (o, lt, r) in enumerate(group):
                        nc.tensor.matmul(
                            out=o,
                            lhsT=lt,
                            rhs=r,
                            start=(k == 0),
                            stop=(k == len(group) - 1),
                        )

                # top-16 (largest score == nearest): 8 + 8 on DVE, over PSUM
                idx16 = idxp.tile([128, K_G], U32, tag="idx")
                m8 = idxp.tile([128, 8], F32, tag="m8")
                nc.vector.max(out=m8[:], in_=ps[:])
                nc.vector.max_index(
                    out=idx16[:, 0:8], in_max=m8[:], in_values=ps[:]
                )
                # first half of the gathers can start as soon as the first
                # 8 indices are known
                nbr = nbrp.tile([128, K_G, C], F16, tag="nbr")
                for s in range(8):
                    nc.gpsimd.indirect_dma_start(
                        out=nbr[:, s, :],
                        out_offset=None,
                        in_=xn_rows[bi][:],
                        in_offset=IndirectOffsetOnAxis(
                            ap=idx16[:, s:s + 1], axis=0
                        ),
                    )
                nc.vector.match_replace(
                    out=ps[:],
                    in_to_replace=m8[:],
                    in_values=ps[:],
                    imm_value=float(-BIG),
                )
                m8b = idxp.tile([128, 8], F32, tag="m8b")
                nc.vector.max(out=m8b[:], in_=ps[:])
                nc.vector.max_index(
                    out=idx16[:, 8:16], in_max=m8b[:], in_values=ps[:]
                )
                if dbg:
                    nc.sync.dma_start(out=idx_dump[bi, rb], in_=idx16[:])

                for s in range(8, K_G):
                    nc.gpsimd.indirect_dma_start(
                        out=nbr[:, s, :],
                        out_offset=None,
                        in_=xn_rows[bi][:],
                        in_offset=IndirectOffsetOnAxis(
                            ap=idx16[:, s:s + 1], axis=0
                        ),
                    )
                if dbg:
                    nc.sync.dma_start(out=nbr_dump[bi, rb], in_=nbr[:])
                nbr_t[rb] = nbr

            def stage_b(rb, bi=bi, xn_nc=xn_nc, md_cn=md_cn, nbr_t=nbr_t):
                rbs = slice(rb * 128, (rb + 1) * 128)
                nbr = nbr_t.pop(rb)
                # min/max over the 16 neighbors (TT trees on DVE, fp16 2x)
                tmax = treep.tile([128, K_G // 2, C], F16, tag="tmax")
                tmin = treep.tile([128, K_G // 2, C], F16, tag="tmin")
                nc.vector.tensor_tensor(
                    tmax[:], nbr[:, 0:8, :], nbr[:, 8:16, :], op=AluOpType.max
                )
                nc.vector.tensor_tensor(
                    tmin[:], nbr[:, 0:8, :], nbr[:, 8:16, :], op=AluOpType.min
                )
                w_ = 4
                while w_ >= 1:
                    nc.vector.tensor_tensor(
                        tmax[:, 0:w_, :],
                        tmax[:, 0:w_, :],
                        tmax[:, w_:2 * w_, :],
                        op=AluOpType.max,
                    )
                    nc.vector.tensor_tensor(
                        tmin[:, 0:w_, :],
                        tmin[:, 0:w_, :],
                        tmin[:, w_:2 * w_, :],
                        op=AluOpType.min,
                    )
                    w_ //= 2

                # md = max(xn - min, max - xn)
                md_nc = mdncp.tile([128, C], F16, tag="md_nc")
                d1 = mdncp.tile([128, C], F16, tag="d1")
                nc.vector.tensor_tensor(
                    d1[:], xn_nc[:, rb, :], tmin[:, 0, :], op=AluOpType.subtract
                )
                nc.vector.tensor_tensor(
                    md_nc[:], tmax[:, 0, :], xn_nc[:, rb, :],
                    op=AluOpType.subtract,
                )
                nc.vector.tensor_tensor(
                    md_nc[:], md_nc[:], d1[:], op=AluOpType.max
                )

                # transpose md block into (C, N) tiles (PE + ACT copy)
                for ct in range(CT):
                    ps = ps_tp.tile([128, 128], F16, tag="tp")
                    nc.tensor.transpose(
                        out=ps[:],
                        in_=md_nc[:, ct * 128:(ct + 1) * 128],
                        identity=ident_h[:],
                    )
                    nc.scalar.copy(md_cn[ct][:, rbs], ps[:])

            # 2-stage software pipeline: trees(rb-2) issue after scans(rb)
            # so the in-order DVE never stalls on the gather latency; ready
            # conv pieces (previous batch, then this batch's h0 half once
            # rbs 0-3 are emitted) slot in to spread PSUM/ACT load
            for rb in range(NB):
                stage_a(rb)
                if rb >= 3:
                    stage_b(rb - 3)
                if rb == 6:
                    pending_conv.extend(my_h0_pieces)
                for _ in range(2):
                    if pending_conv:
                        pending_conv.pop(0)()
            for rb in range(NB - 3, NB):
                stage_b(rb)
                if pending_conv:
                    pending_conv.pop(0)()
            pending_conv.extend(my_h1_pieces)

            if dbg:
                for ct in range(CT):
                    nc.sync.dma_start(
                        out=md_dump[bi, ct * 128:(ct + 1) * 128, :],
                        in_=md_cn[ct][:],
                    )

        while pending_conv:
            pending_conv.pop(0)()

        # ---- BN stats: reduce partials, all-reduce across cores ----
        stats_sb = statp.tile([128, 2 * OT], F32)
        nc.vector.tensor_reduce(
            stats_sb[:, 0:OT],
            part_s1[:].rearrange("p (o q) -> p o q", q=B_LOC * NH),
            axis=AX,
            op=AluOpType.add,
        )
        nc.vector.tensor_reduce(
            stats_sb[:, OT:2 * OT],
            part_s2[:].rearrange("p (o q) -> p o q", q=B_LOC * NH),
            axis=AX,
            op=AluOpType.add,
        )
        if dbg:
            nc.sync.dma_start(out=s_dump[:, 0:OT * B_LOC], in_=part_s1[:])
            nc.sync.dma_start(
                out=s_dump[:, OT * B_LOC:2 * OT * B_LOC], in_=part_s2[:]
            )
        nc.gpsimd.dma_start(out=stats_in[:, :], in_=stats_sb[:])
        if collective:
            nc.gpsimd.collective_compute(
                "AllReduce",
                AluOpType.add,
                replica_groups=[list(range(N_CORES))],
                ins=[stats_in.ap().opt()],
                outs=[stats_out.ap().opt()],
            )
        else:
            # sim-only stand-in: single-core timing proxy for the 4KB AR
            nc.gpsimd.dma_start(out=stats_out[:, :], in_=stats_in[:, :])
        stats_red = statp.tile([128, 2 * OT], F32)
        nc.gpsimd.dma_start(out=stats_red[:], in_=stats_out[:, :])

        # mean/var/affine (per channel; channel c = partition p, col ot)
        inv_cnt = 1.0 / float(B * N)
        mean4 = statp.tile([128, OT], F32)
        nc.vector.tensor_scalar_mul(mean4[:], stats_red[:, 0:OT], inv_cnt)
        var4 = statp.tile([128, OT], F32)
        # var = s2/cnt - mean^2
        nc.vector.tensor_scalar_mul(var4[:], stats_red[:, OT:2 * OT], inv_cnt)
        msq = statp.tile([128, OT], F32)
        nc.vector.tensor_tensor(msq[:], mean4[:], mean4[:], op=AluOpType.mult)
        nc.vector.tensor_tensor(var4[:], var4[:], msq[:], op=AluOpType.subtract)
        # rstd = 1/sqrt(var+eps)
        nc.vector.tensor_scalar_add(var4[:], var4[:], float(BN_EPS))
        std4 = statp.tile([128, OT], F32)
        nc.scalar.activation(std4[:], var4[:], AF.Sqrt)
        rstd4 = statp.tile([128, OT], F32)
        nc.vector.reciprocal(rstd4[:], std4[:])
        a4 = statp.tile([128, OT], F32)
        nc.vector.tensor_tensor(a4[:], gamma4[:], rstd4[:], op=AluOpType.mult)
        b4 = statp.tile([128, OT], F32)
        # b4 = beta - mean * a
        nc.vector.scalar_tensor_tensor(
            out=b4[:],
            in0=mean4[:],
            scalar=-1.0,
            in1=a4[:],
            op0=AluOpType.mult,
            op1=AluOpType.mult,
        )
        nc.vector.tensor_tensor(b4[:], b4[:], beta4[:], op=AluOpType.add)

        # ---- fused BN + exact gelu on ACT (f32 out), then store ----
        for bi in range(B_LOC):
            for ot in range(OT):
                yt = y_tiles[(bi, ot)]
                ot_f32 = outp.tile([128, N], F16, tag="of")
                nc.scalar.activation(
                    ot_f32[:],
                    yt[:],
                    AF.Gelu if use_gelu else AF.Copy,
                    bias=b4[:, ot:ot + 1] if use_gelu else 0.0,
                    scale=a4[:, ot:ot + 1],
                )
                nc.sync.dma_start(
                    out=out_dram[bi, ot * 128:(ot + 1) * 128, :], in_=ot_f32[:]
                )

    nc.compile()
    return nc


_NC_CACHE = None


def _get_nc():
    global _NC_CACHE
    if _NC_CACHE is None:
        _NC_CACHE = build_kernel()
    return _NC_CACHE


def _prep_shared(w, gamma, beta):
    w = np.asarray(w, np.float32)
    wev = np.ascontiguousarray(w[:, 0::2].T.astype(np.float16))  # (C, OUT)
    wod = np.ascontiguousarray(w[:, 1::2].T.astype(np.float16))
    gamma4 = np.ascontiguousarray(
        np.asarray(gamma, np.float32).reshape(OT, 128).T
    )
    beta4 = np.ascontiguousarray(np.asarray(beta, np.float32).reshape(OT, 128).T)
    return wev, wod, gamma4, beta4


def kernel(x, w, b, gamma, beta):
    x = np.ascontiguousarray(np.asarray(x, np.float32))
    assert x.shape == (B, C, N), x.shape
    wev, wod, gamma4, beta4 = _prep_shared(w, gamma, beta)
    # b cancels exactly in training-mode BN (see module docstring).
    nc = _get_nc()
    in_maps = [
        {
            "x": np.ascontiguousarray(x[c * B_LOC:(c + 1) * B_LOC]),
            "wev": wev,
            "wod": wod,
            "gamma4": gamma4,
            "beta4": beta4,
        }
        for c in range(N_CORES)
    ]
    res = run_bass_kernel_spmd(nc, in_maps, core_ids=list(range(N_CORES)))
    out = np.concatenate([res.results[c]["out"] for c in range(N_CORES)], axis=0)
    return out[..., None].astype(np.float32)


# revision 60
# speedup vs baseline: 1.0274x; 1.0274x over previous
"""Trainium2 Bass kernel for nn_MaxGraphConv (gnn_message_passing).

Reference computation (per batch element, all f32):
  xn   = L2-normalize(x^T along C)                       # (N, C)
  d2   = |xn_i - xn_j|^2 via Gram matrix, self excluded
  idx  = 16 nearest neighbors per point (smallest d2)
  md_c = max_k |xn_ic - xn_jc| over the 16 neighbors      # (N, C)
  feat = interleave(xn, md) -> (2C, N); y = W @ feat + b
  y    = BatchNorm(training stats over (B, N)) ; out = gelu_exact(y)

Sharding: data-parallel over B across 8 cores (2 batches/core); conv/BN
params replicated; BN statistics all-reduced (4KB) on device.

Device algorithm per batch (fp16 compute, f32 norms/scores/stats):
  * Since xn is L2-normalized, |xn_m|^2 == 1, so ranking by distance is
    ranking by the raw Gram G = xn^T xn DESCENDING. No score correction.
  * Gram via PE fp16 matmuls into a 2-bank PSUM tile; the diagonal is
    suppressed by accumulating -60000*I with one extra PE matmul.
  * top-16 via DVE InstMax/InstMaxIndex/InstMatchReplace (8 + 8) run
    directly on the f32 PSUM scores (no SBUF copy, exact selection).
  * (N, C)-layout xn via DMA-transpose XBAR (16-bit), written once to
    DRAM as the gather source; the 16 neighbor rows per point fetched
    with one indirect DMA per neighbor slot ([128,1] offsets; the
    first 8 issue as soon as the first max_index lands).
  * md from min/max TT trees over the 16 gathered rows (DVE, fp16 2x);
    md = max(xn - min, max - xn).
  * conv as W_even @ xn + W_odd @ md (W pre-split+transposed on host)
    in per-(out-tile, half) pieces interleaved into the next batch's
    row-block loop; the row-block loop itself is 3-stage
    software-pipelined so the in-order DVE never waits on gathers.
  * BN: per-channel sum/sumsq (ACT accum during PSUM->SBUF copy and a
    Square pass) -> 4KB AllReduce -> affine+gelu fused on ACT.
  * conv bias b cancels exactly in training-mode BN (y+b shifts the mean
    by b) so it is accepted and ignored.
"""

import sys

if "/opt/trn_rl_repo" not in sys.path:
    sys.path.insert(0, "/opt/trn_rl_repo")

import numpy as np

import concourse.bacc as bacc
import concourse.mybir as mybir
import concourse.tile as tile
from concourse import bass
from concourse.alu_op_type import AluOpType
from concourse.bass import IndirectOffsetOnAxis
from concourse.bass_utils import run_bass_kernel_spmd
from concourse.masks import make_identity

F32 = mybir.dt.float32
F16 = mybir.dt.float16
U32 = mybir.dt.uint32
U16 = mybir.dt.uint16
I16 = mybir.dt.int16
AF = mybir.ActivationFunctionType
AX = None  # set lazily (bass_rust.AxisListType.X)

N_CORES = 8
B, C, N = 16, 256, 1024
B_LOC = B // N_CORES          # 2 batches per core
OUT = 2 * C                   # 512
K_G = 16
BN_EPS = 1e-5
BIG = 1.0e30                  # f32 "infinity" for masking PSUM scores
NB = N // 128                 # 8 row blocks per batch
CT = C // 128                 # 2 channel tiles
OT = OUT // 128               # 4 out-channel tiles
NH = N // 512                 # 2 free-dim halves for matmul


def build_kernel(use_gelu=True, collective=True, dbg=False):
    import bass_rust

    global AX
    AX = bass_rust.AxisListType.X

    nc = bacc.Bacc("TRN2", target_bir_lowering=False, debug=False)

    x_in = nc.dram_tensor("x", [B_LOC, C, N], F32, kind="ExternalInput")
    wev_in = nc.dram_tensor("wev", [C, OUT], F16, kind="ExternalInput")
    wod_in = nc.dram_tensor("wod", [C, OUT], F16, kind="ExternalInput")
    gamma_in = nc.dram_tensor("gamma4", [128, OT], F32, kind="ExternalInput")
    beta_in = nc.dram_tensor("beta4", [128, OT], F32, kind="ExternalInput")
    out_dram = nc.dram_tensor("out", [B_LOC, OUT, N], F16, kind="ExternalOutput")

    # gather sources (offset-0 requirement for indirect DMA src)
    xn_rows = [nc.dram_tensor(f"xn_rows{bi}", [N, C], F16) for bi in range(B_LOC)]
    stats_in = nc.dram_tensor("stats_in", [128, 2 * OT], F32)
    stats_out = nc.dram_tensor("stats_out", [128, 2 * OT], F32)
    if dbg:
        idx_dump = nc.dram_tensor(
            "idx_dump", [B_LOC, NB, 128, K_G], U32, kind="ExternalOutput"
        )
        md_dump = nc.dram_tensor(
            "md_dump", [B_LOC, CT * 128, N], F16, kind="ExternalOutput"
        )
        nbr_dump = nc.dram_tensor(
            "nbr_dump", [B_LOC, NB, 128, K_G, C], F16, kind="ExternalOutput"
        )
        y_dump = nc.dram_tensor(
            "y_dump", [B_LOC, OUT, N], F16, kind="ExternalOutput"
        )
        s_dump = nc.dram_tensor(
            "s_dump", [128, 4 * OT], F32, kind="ExternalOutput"
        )

    from contextlib import ExitStack

    with tile.TileContext(nc) as tc, ExitStack() as ctx:
        ep = ctx.enter_context
        constp = ep(tc.tile_pool(name="const", bufs=1))
        wpool = ep(tc.tile_pool(name="wpool", bufs=CT))
        xload = ep(tc.tile_pool(name="xload", bufs=2))
        sqp = ep(tc.tile_pool(name="sqp", bufs=2))
        rowp = ep(tc.tile_pool(name="rowp", bufs=2))
        bcp = ep(tc.tile_pool(name="bcp", bufs=2))
        xnp = ep(tc.tile_pool(name="xnp", bufs=2 * CT))
        xnncp = ep(tc.tile_pool(name="xnnc", bufs=2))
        idxp = ep(tc.tile_pool(name="idxp", bufs=6))
        scp = ep(tc.tile_pool(name="scp", bufs=3))
        nbrp = ep(tc.tile_pool(name="nbrp", bufs=4))
        treep = ep(tc.tile_pool(name="treep", bufs=2))
        mdncp = ep(tc.tile_pool(name="mdnc", bufs=4))
        mdcnp = ep(tc.tile_pool(name="mdcn", bufs=CT))
        ypool = ep(tc.tile_pool(name="ypool", bufs=2 * OT))
        outp = ep(tc.tile_pool(name="outp", bufs=2))
        statp = ep(tc.tile_pool(name="statp", bufs=1))
        ps_tp = ep(tc.tile_pool(name="ps_tp", bufs=2, space="PSUM"))
        ps_mm = ep(tc.tile_pool(name="ps_mm", bufs=2, space="PSUM"))

        # ---- constants ----
        ident_h = constp.tile([128, 128], F16)
        make_identity(nc, ident_h[:])
        ones_col = constp.tile([128, 1], F32)
        nc.vector.memset(ones_col[:], 1.0)

        # per-channel partial sums of y and y^2: col = (ot*2 + bi)*2 + h
        part_s1 = statp.tile([128, OT * B_LOC * NH], F32)
        part_s2 = statp.tile([128, OT * B_LOC * NH], F32)

        y_tiles = {}  # (bi, ot) -> tile (128, N) fp16

        # ---- phase 1 (both batches): load, norms, normalize, layouts ----
        xn_ct_b = {}
        xn_nc_b = {}
        for bi in range(B_LOC):
            x_ct = [
                xload.tile([128, N], F32, tag="x", name=f"x{bi}_{ct}")
                for ct in range(CT)
            ]
            xsq_ct = [
                sqp.tile([128, N], F32, tag="xsq", name=f"xsq{bi}_{ct}")
                for ct in range(CT)
            ]
            # chunk loads and squares by half so the norm chain starts early
            for h in range(NH):
                hs = slice(h * 512, (h + 1) * 512)
                for ct in range(CT):
                    nc.sync.dma_start(
                        out=x_ct[ct][:, hs],
                        in_=x_in[bi, ct * 128:(ct + 1) * 128, hs],
                    )
                    nc.scalar.activation(
                        xsq_ct[ct][:, hs], x_ct[ct][:, hs], AF.Square
                    )

            rnorm_row = rowp.tile([1, N], F32, tag="rnorm")
            for h in range(NH):
                hs = slice(h * 512, (h + 1) * 512)
                ps = ps_mm.tile([1, 512], F32, tag="cv", name=f"srow{bi}_{h}")
                for ct in range(CT):
                    nc.tensor.matmul(
                        out=ps[:],
                        lhsT=ones_col[:],
                        rhs=xsq_ct[ct][:, hs],
                        start=(ct == 0),
                        stop=(ct == CT - 1),
                    )
                # sqrt on ACT (PSUM -> SBUF), then reciprocal on DVE
                srt = rowp.tile([1, 512], F32, tag="srt")
                nc.scalar.activation(srt[:], ps[:], AF.Sqrt)
                nc.vector.reciprocal(rnorm_row[:, hs], srt[:])

            # broadcast row across partitions (GpSimd custom op)
            rnorm_bc = bcp.tile([128, N], F32, tag="rnorm_bc")
            nc.gpsimd.partition_broadcast(rnorm_bc[:], rnorm_row[:])

            # normalize: xn = x * rnorm (column-wise), fp16 out
            xn_ct = []
            for ct in range(CT):
                t = xnp.tile([128, N], F16, tag="xn")
                nc.vector.tensor_tensor(
                    t[:], x_ct[ct][:], rnorm_bc[:], op=AluOpType.mult
                )
                xn_ct.append(t)
            xn_ct_b[bi] = xn_ct

            # (N, C) layout via DMA-transpose XBAR; gather rows to DRAM
            xn_nc = xnncp.tile([128, NB, C], F16, tag="xn_nc")
            for ct in range(CT):
                nc.sync.dma_start(
                    out=xn_nc[:, :, ct * 128:(ct + 1) * 128],
                    in_=xn_ct[ct][:],
                    transpose=True,
                )
            nc.sync.dma_start(
                out=xn_rows[bi][:, :].rearrange("(rb p) c -> p rb c", p=128),
                in_=xn_nc[:],
            )
            xn_nc_b[bi] = xn_nc

        # ---- replicated weights / BN params (needed late; after x loads
        # so batch 0's norm chain leads the DMA queue) ----
        nbig_h = constp.tile([128, 128], F16)
        nc.scalar.activation(nbig_h[:], ident_h[:], AF.Copy, scale=-60000.0)
        wev = []
        wod = []
        for ct in range(CT):
            t = wpool.tile([128, OUT], F16, tag="wev", name=f"wev{ct}")
            nc.sync.dma_start(out=t[:], in_=wev_in[ct * 128:(ct + 1) * 128, :])
            wev.append(t)
            t = wpool.tile([128, OUT], F16, tag="wod", name=f"wod{ct}")
            nc.sync.dma_start(out=t[:], in_=wod_in[ct * 128:(ct + 1) * 128, :])
            wod.append(t)
        gamma4 = constp.tile([128, OT], F32)
        nc.sync.dma_start(out=gamma4[:], in_=gamma_in[:, :])
        beta4 = constp.tile([128, OT], F32)
        nc.sync.dma_start(out=beta4[:], in_=beta_in[:, :])

        # ---- per batch: row blocks (software-pipelined) then conv ----
        for bi in range(B_LOC):
            xn_ct = xn_ct_b[bi]
            xn_nc = xn_nc_b[bi]

            if bi == 0:
                pending_conv = []

            md_cn = []
            for ct in range(CT):
                md_cn.append(
                    mdcnp.tile([128, N], F16, tag="md_cn", name=f"md_cn{bi}_{ct}")
                )

            nbr_t = {}
            def conv_piece(ot, h, bi=bi, xn_ct=xn_ct, md_cn=md_cn):
                ots = slice(ot * 128, (ot + 1) * 128)
                if h == 0:
                    yt = ypool.tile(
                        [128, N], F16, tag="y", name=f"y{bi}_{ot}"
                    )
                    y_tiles[(bi, ot)] = yt
                else:
                    yt = y_tiles[(bi, ot)]
                hs = slice(h * 512, (h + 1) * 512)
                ps = ps_mm.tile(
                    [128, 512], F32, tag="cv", name=f"cps{bi}_{ot}_{h}"
                )
                for ct in range(CT):
                    nc.tensor.matmul(
                        out=ps[:],
                        lhsT=wev[ct][:, ots],
                        rhs=xn_ct[ct][:, hs],
                        start=(ct == 0),
                        stop=False,
                    )
                for ct in range(CT):
                    nc.tensor.matmul(
                        out=ps[:],
                        lhsT=wod[ct][:, ots],
                        rhs=md_cn[ct][:, hs],
                        start=False,
                        stop=(ct == CT - 1),
                    )
                # move PSUM->SBUF on ACT with fused per-channel sum
                col = (ot * B_LOC + bi) * NH + h
                nc.scalar.activation(
                    yt[:, hs],
                    ps[:],
                    AF.Copy,
                    accum_out=part_s1[:, col:col + 1],
                )
                # sumsq with fused per-channel sum; the final batch's h1
                # pieces flush at the tail where ACT is the critical chain,
                # so run those squares on the then-idle DVE instead
                sq_scr = sqp.tile(
                    [128, 512], F16, tag="ysq", name=f"ysq{bi}_{ot}_{h}"
                )
                if bi == B_LOC - 1 and h == NH - 1:
                    nc.vector.scalar_tensor_tensor(
                        out=sq_scr[:],
                        in0=yt[:, hs],
                        scalar=1.0,
                        in1=yt[:, hs],
                        op0=AluOpType.mult,
                        op1=AluOpType.mult,
                        accum_out=part_s2[:, col:col + 1],
                    )
                else:
                    nc.scalar.activation(
                        sq_scr[:],
                        yt[:, hs],
                        AF.Square,
                        accum_out=part_s2[:, col:col + 1],
                    )
                if dbg and h == NH - 1:
                    nc.sync.dma_start(
                        out=y_dump[bi, ot * 128:(ot + 1) * 128, :], in_=yt[:]
                    )


            my_h0_pieces = [
                (lambda ot=ot, f=conv_piece: f(ot, 0)) for ot in range(OT)
            ]
            my_h1_pieces = [
                (lambda ot=ot, f=conv_piece: f(ot, 1)) for ot in range(OT)
            ]


            def stage_a(rb, bi=bi, xn_ct=xn_ct):
                rbs = slice(rb * 128, (rb + 1) * 128)
                # Gram row block straight into a 2-bank PSUM tile; the top-k
                # scans read PSUM directly (f32 scores, no SBUF copy).
                ps = ps_mm.tile([128, N], F32, tag="mm")
                for h in range(NH):
                    hs = slice(h * 512, (h + 1) * 512)
                    # matmul group per half: 2 channel tiles, plus (for the
                    # half containing the diagonal) a -60000*I accumulation
                    # for self-exclusion (frees a DVE op per row block)
                    group = [
                        (ps[:, hs], xn_ct[ct][:, rbs], xn_ct[ct][:, hs])
                        for ct in range(CT)
                    ]
                    if h == rb // (NB // NH):
                        group.append(
                            (
                                ps[:, rb * 128:(rb + 1) * 128],
                                nbig_h[:],
                                ident_h[:],
                            )
                        )
                    for k, (o, lt, r) in enumerate(group):
                        nc.tensor.matmul(
                            out=o,
                            lhsT=lt,
                            rhs=r,
                            start=(k == 0),
                            stop=(k == len(group) - 1),
                        )

                # scores move to SBUF f32 on ACT: releases the PSUM tile
                # early and avoids the PSUM-access penalty on each DVE scan
                sc = scp.tile([128, N], F32, tag="sc")
                nc.scalar.copy(sc[:], ps[:])

                # top-16 (largest score == nearest): 8 + 8 on DVE
                idx16 = idxp.tile([128, K_G], U32, tag="idx")
                m8 = idxp.tile([128, 8], F32, tag="m8")
                nc.vector.max(out=m8[:], in_=sc[:])
                nc.vector.max_index(
                    out=idx16[:, 0:8], in_max=m8[:], in_values=sc[:]
                )
                # first half of the gathers can start as soon as the first
                # 8 indices are known
                nbr = nbrp.tile([128, K_G, C], F16, tag="nbr")
                for s in range(8):
                    nc.gpsimd.indirect_dma_start(
                        out=nbr[:, s, :],
                        out_offset=None,
                        in_=xn_rows[bi][:],
                        in_offset=IndirectOffsetOnAxis(
                            ap=idx16[:, s:s + 1], axis=0
                        ),
                    )
                nc.vector.match_replace(
                    out=sc[:],
                    in_to_replace=m8[:],
                    in_values=sc[:],
                    imm_value=float(-BIG),
                )
                m8b = idxp.tile([128, 8], F32, tag="m8b")
                nc.vector.max(out=m8b[:], in_=sc[:])
                nc.vector.max_index(
                    out=idx16[:, 8:16], in_max=m8b[:], in_values=sc[:]
                )
                if dbg:
                    nc.sync.dma_start(out=idx_dump[bi, rb], in_=idx16[:])

                for s in range(8, K_G):
                    nc.gpsimd.indirect_dma_start(
                        out=nbr[:, s, :],
                        out_offset=None,
                        in_=xn_rows[bi][:],
                        in_offset=IndirectOffsetOnAxis(
                            ap=idx16[:, s:s + 1], axis=0
                        ),
                    )
                if dbg:
                    nc.sync.dma_start(out=nbr_dump[bi, rb], in_=nbr[:])
                nbr_t[rb] = nbr

            def stage_b(rb, bi=bi, xn_nc=xn_nc, md_cn=md_cn, nbr_t=nbr_t):
                rbs = slice(rb * 128, (rb + 1) * 128)
                nbr = nbr_t.pop(rb)
                # min/max over the 16 neighbors (TT trees on DVE, fp16 2x)
                tmax = treep.tile([128, K_G // 2, C], F16, tag="tmax")
                tmin = treep.tile([128, K_G // 2, C], F16, tag="tmin")
                nc.vector.tensor_tensor(
                    tmax[:], nbr[:, 0:8, :], nbr[:, 8:16, :], op=AluOpType.max
                )
                nc.vector.tensor_tensor(
                    tmin[:], nbr[:, 0:8, :], nbr[:, 8:16, :], op=AluOpType.min
                )
                w_ = 4
                while w_ >= 1:
                    nc.vector.tensor_tensor(
                        tmax[:, 0:w_, :],
                        tmax[:, 0:w_, :],
                        tmax[:, w_:2 * w_, :],
                        op=AluOpType.max,
                    )
                    nc.vector.tensor_tensor(
                        tmin[:, 0:w_, :],
                        tmin[:, 0:w_, :],
                        tmin[:, w_:2 * w_, :],
                        op=AluOpType.min,
                    )
                    w_ //= 2

                # md = max(xn - min, max - xn)
                md_nc = mdncp.tile([128, C], F16, tag="md_nc")
                d1 = mdncp.tile([128, C], F16, tag="d1")
                nc.vector.tensor_tensor(
                    d1[:], xn_nc[:, rb, :], tmin[:, 0, :], op=AluOpType.subtract
                )
                nc.vector.tensor_tensor(
                    md_nc[:], tmax[:, 0, :], xn_nc[:, rb, :],
                    op=AluOpType.subtract,
                )
                nc.vector.tensor_tensor(
                    md_nc[:], md_nc[:], d1[:], op=AluOpType.max
                )

                # transpose md block into (C, N) tiles (PE + ACT copy)
                for ct in range(CT):
                    ps = ps_tp.tile([128, 128], F16, tag="tp")
                    nc.tensor.transpose(
                        out=ps[:],
                        in_=md_nc[:, ct * 128:(ct + 1) * 128],
                        identity=ident_h[:],
                    )
                    nc.scalar.copy(md_cn[ct][:, rbs], ps[:])

            # 2-stage software pipeline: trees(rb-2) issue after scans(rb)
            # so the in-order DVE never stalls on the gather latency; ready
            # conv pieces (previous batch, then this batch's h0 half once
            # rbs 0-3 are emitted) slot in to spread PSUM/ACT load
            for rb in range(NB):
                stage_a(rb)
                if rb >= 3:
                    stage_b(rb - 3)
                if rb == 6:
                    pending_conv.extend(my_h0_pieces)
                for _ in range(2):
                    if pending_conv:
                        pending_conv.pop(0)()
            for rb in range(NB - 3, NB):
                stage_b(rb)
                if pending_conv:
                    pending_conv.pop(0)()
            pending_conv.extend(my_h1_pieces)

            if dbg:
                for ct in range(CT):
                    nc.sync.dma_start(
                        out=md_dump[bi, ct * 128:(ct + 1) * 128, :],
                        in_=md_cn[ct][:],
                    )

        while pending_conv:
            pending_conv.pop(0)()

        # ---- BN stats: reduce partials, all-reduce across cores ----
        stats_sb = statp.tile([128, 2 * OT], F32)
        nc.vector.tensor_reduce(
            stats_sb[:, 0:OT],
            part_s1[:].rearrange("p (o q) -> p o q", q=B_LOC * NH),
            axis=AX,
            op=AluOpType.add,
        )
        nc.vector.tensor_reduce(
            stats_sb[:, OT:2 * OT],
            part_s2[:].rearrange("p (o q) -> p o q", q=B_LOC * NH),
            axis=AX,
            op=AluOpType.add,
        )
        if dbg:
            nc.sync.dma_start(out=s_dump[:, 0:OT * B_LOC], in_=part_s1[:])
            nc.sync.dma_start(
                out=s_dump[:, OT * B_LOC:2 * OT * B_LOC], in_=part_s2[:]
            )
        nc.gpsimd.dma_start(out=stats_in[:, :], in_=stats_sb[:])
        if collective:
            nc.gpsimd.collective_compute(
                "AllReduce",
                AluOpType.add,
                replica_groups=[list(range(N_CORES))],
                ins=[stats_in.ap().opt()],
                outs=[stats_out.ap().opt()],
            )
        else:
            # sim-only stand-in: single-core timing proxy for the 4KB AR
            nc.gpsimd.dma_start(out=stats_out[:, :], in_=stats_in[:, :])
        stats_red = statp.tile([128, 2 * OT], F32)
        nc.gpsimd.dma_start(out=stats_red[:], in_=stats_out[:, :])

        # mean/var/affine (per channel; channel c = partition p, col ot)
        inv_cnt = 1.0 / float(B * N)
        mean4 = statp.tile([128, OT], F32)
        nc.vector.tensor_scalar_mul(mean4[:], stats_red[:, 0:OT], inv_cnt)
        var4 = statp.tile([128, OT], F32)
        # var = s2/cnt - mean^2
        nc.vector.tensor_scalar_mul(var4[:], stats_red[:, OT:2 * OT], inv_cnt)
        msq = statp.tile([128, OT], F32)
        nc.vector.tensor_tensor(msq[:], mean4[:], mean4[:], op=AluOpType.mult)
        nc.vector.tensor_tensor(var4[:], var4[:], msq[:], op=AluOpType.subtract)
        # rstd = 1/sqrt(var+eps)
        nc.vector.tensor_scalar_add(var4[:], var4[:], float(BN_EPS))
        std4 = statp.tile([128, OT], F32)
        nc.scalar.activation(std4[:], var4[:], AF.Sqrt)
        rstd4 = statp.tile([128, OT], F32)
        nc.vector.reciprocal(rstd4[:], std4[:])
        a4 = statp.tile([128, OT], F32)
        nc.vector.tensor_tensor(a4[:], gamma4[:], rstd4[:], op=AluOpType.mult)
        b4 = statp.tile([128, OT], F32)
        # b4 = beta - mean * a
        nc.vector.scalar_tensor_tensor(
            out=b4[:],
            in0=mean4[:],
            scalar=-1.0,
            in1=a4[:],
            op0=AluOpType.mult,
            op1=AluOpType.mult,
        )
        nc.vector.tensor_tensor(b4[:], b4[:], beta4[:], op=AluOpType.add)

        # ---- fused BN + exact gelu on ACT (f32 out), then store ----
        for bi in range(B_LOC):
            for ot in range(OT):
                yt = y_tiles[(bi, ot)]
                ot_f32 = outp.tile([128, N], F16, tag="of")
                nc.scalar.activation(
                    ot_f32[:],
                    yt[:],
                    AF.Gelu if use_gelu else AF.Copy,
                    bias=b4[:, ot:ot + 1] if use_gelu else 0.0,
                    scale=a4[:, ot:ot + 1],
                )
                nc.sync.dma_start(
                    out=out_dram[bi, ot * 128:(ot + 1) * 128, :], in_=ot_f32[:]
                )

    nc.compile()
    return nc


_NC_CACHE = None


def _get_nc():
    global _NC_CACHE
    if _NC_CACHE is None:
        _NC_CACHE = build_kernel()
    return _NC_CACHE


def _prep_shared(w, gamma, beta):
    w = np.asarray(w, np.float32)
    wev = np.ascontiguousarray(w[:, 0::2].T.astype(np.float16))  # (C, OUT)
    wod = np.ascontiguousarray(w[:, 1::2].T.astype(np.float16))
    gamma4 = np.ascontiguousarray(
        np.asarray(gamma, np.float32).reshape(OT, 128).T
    )
    beta4 = np.ascontiguousarray(np.asarray(beta, np.float32).reshape(OT, 128).T)
    return wev, wod, gamma4, beta4


def kernel(x, w, b, gamma, beta):
    x = np.ascontiguousarray(np.asarray(x, np.float32))
    assert x.shape == (B, C, N), x.shape
    wev, wod, gamma4, beta4 = _prep_shared(w, gamma, beta)
    # b cancels exactly in training-mode BN (see module docstring).
    nc = _get_nc()
    in_maps = [
        {
            "x": np.ascontiguousarray(x[c * B_LOC:(c + 1) * B_LOC]),
            "wev": wev,
            "wod": wod,
            "gamma4": gamma4,
            "beta4": beta4,
        }
        for c in range(N_CORES)
    ]
    res = run_bass_kernel_spmd(nc, in_maps, core_ids=list(range(N_CORES)))
    out = np.concatenate([res.results[c]["out"] for c in range(N_CORES)], axis=0)
    return out[..., None].astype(np.float32)


# revision 62
# speedup vs baseline: 1.0406x; 1.0128x over previous
"""Trainium2 Bass kernel for nn_MaxGraphConv (gnn_message_passing).

Reference computation (per batch element, all f32):
  xn   = L2-normalize(x^T along C)                       # (N, C)
  d2   = |xn_i - xn_j|^2 via Gram matrix, self excluded
  idx  = 16 nearest neighbors per point (smallest d2)
  md_c = max_k |xn_ic - xn_jc| over the 16 neighbors      # (N, C)
  feat = interleave(xn, md) -> (2C, N); y = W @ feat + b
  y    = BatchNorm(training stats over (B, N)) ; out = gelu_exact(y)

Sharding: data-parallel over B across 8 cores (2 batches/core); conv/BN
params replicated; BN statistics all-reduced (4KB) on device.

Device algorithm per batch (fp16 compute, f32 norms/scores/stats):
  * Since xn is L2-normalized, |xn_m|^2 == 1, so ranking by distance is
    ranking by the raw Gram G = xn^T xn DESCENDING. No score correction.
  * Gram via PE fp16 matmuls into a 2-bank PSUM tile; the diagonal is
    suppressed by accumulating -60000*I with one extra PE matmul.
  * top-16 via DVE InstMax/InstMaxIndex/InstMatchReplace (8 + 8) run
    directly on the f32 PSUM scores (no SBUF copy, exact selection).
  * (N, C)-layout xn via DMA-transpose XBAR (16-bit), written once to
    DRAM as the gather source; the 16 neighbor rows per point fetched
    with one indirect DMA per neighbor slot ([128,1] offsets; the
    first 8 issue as soon as the first max_index lands).
  * md from min/max TT trees over the 16 gathered rows (DVE, fp16 2x);
    md = max(xn - min, max - xn).
  * conv as W_even @ xn + W_odd @ md (W pre-split+transposed on host)
    in per-(out-tile, half) pieces interleaved into the next batch's
    row-block loop; the row-block loop itself is 3-stage
    software-pipelined so the in-order DVE never waits on gathers.
  * BN: per-channel sum/sumsq (ACT accum during PSUM->SBUF copy and a
    Square pass) -> 4KB AllReduce -> affine+gelu fused on ACT.
  * conv bias b cancels exactly in training-mode BN (y+b shifts the mean
    by b) so it is accepted and ignored.
"""

import sys

if "/opt/trn_rl_repo" not in sys.path:
    sys.path.insert(0, "/opt/trn_rl_repo")

import numpy as np

import concourse.bacc as bacc
import concourse.mybir as mybir
import concourse.tile as tile
from concourse import bass
from concourse.alu_op_type import AluOpType
from concourse.bass import IndirectOffsetOnAxis
from concourse.bass_utils import run_bass_kernel_spmd
from concourse.masks import make_identity

F32 = mybir.dt.float32
F16 = mybir.dt.float16
U32 = mybir.dt.uint32
U16 = mybir.dt.uint16
I16 = mybir.dt.int16
AF = mybir.ActivationFunctionType
AX = None  # set lazily (bass_rust.AxisListType.X)

N_CORES = 8
B, C, N = 16, 256, 1024
B_LOC = B // N_CORES          # 2 batches per core
OUT = 2 * C                   # 512
K_G = 16
BN_EPS = 1e-5
BIG = 1.0e30                  # f32 "infinity" for masking PSUM scores
NB = N // 128                 # 8 row blocks per batch
CT = C // 128                 # 2 channel tiles
OT = OUT // 128               # 4 out-channel tiles
NH = N // 512                 # 2 free-dim halves for matmul


def build_kernel(use_gelu=True, collective=True, dbg=False):
    import bass_rust

    global AX
    AX = bass_rust.AxisListType.X

    nc = bacc.Bacc("TRN2", target_bir_lowering=False, debug=False)

    x_in = nc.dram_tensor("x", [B_LOC, C, N], F32, kind="ExternalInput")
    wev_in = nc.dram_tensor("wev", [C, OUT], F16, kind="ExternalInput")
    wod_in = nc.dram_tensor("wod", [C, OUT], F16, kind="ExternalInput")
    gamma_in = nc.dram_tensor("gamma4", [128, OT], F32, kind="ExternalInput")
    beta_in = nc.dram_tensor("beta4", [128, OT], F32, kind="ExternalInput")
    out_dram = nc.dram_tensor("out", [B_LOC, OUT, N], F16, kind="ExternalOutput")

    # gather sources (offset-0 requirement for indirect DMA src)
    xn_rows = [nc.dram_tensor(f"xn_rows{bi}", [N, C], F16) for bi in range(B_LOC)]
    stats_in = nc.dram_tensor("stats_in", [128, 2 * OT], F32)
    stats_out = nc.dram_tensor("stats_out", [128, 2 * OT], F32)
    if dbg:
        idx_dump = nc.dram_tensor(
            "idx_dump", [B_LOC, NB, 128, K_G], U32, kind="ExternalOutput"
        )
        md_dump = nc.dram_tensor(
            "md_dump", [B_LOC, CT * 128, N], F16, kind="ExternalOutput"
        )
        nbr_dump = nc.dram_tensor(
            "nbr_dump", [B_LOC, NB, 128, K_G, C], F16, kind="ExternalOutput"
        )
        y_dump = nc.dram_tensor(
            "y_dump", [B_LOC, OUT, N], F16, kind="ExternalOutput"
        )
        s_dump = nc.dram_tensor(
            "s_dump", [128, 4 * OT], F32, kind="ExternalOutput"
        )

    from contextlib import ExitStack

    with tile.TileContext(nc) as tc, ExitStack() as ctx:
        ep = ctx.enter_context
        constp = ep(tc.tile_pool(name="const", bufs=1))
        wpool = ep(tc.tile_pool(name="wpool", bufs=CT))
        xload = ep(tc.tile_pool(name="xload", bufs=2))
        sqp = ep(tc.tile_pool(name="sqp", bufs=2))
        rowp = ep(tc.tile_pool(name="rowp", bufs=2))
        bcp = ep(tc.tile_pool(name="bcp", bufs=2))
        xnp = ep(tc.tile_pool(name="xnp", bufs=2 * CT))
        xnncp = ep(tc.tile_pool(name="xnnc", bufs=2))
        idxp = ep(tc.tile_pool(name="idxp", bufs=6))
        scp = ep(tc.tile_pool(name="scp", bufs=3))
        nbrp = ep(tc.tile_pool(name="nbrp", bufs=4))
        treep = ep(tc.tile_pool(name="treep", bufs=2))
        mdncp = ep(tc.tile_pool(name="mdnc", bufs=4))
        mdcnp = ep(tc.tile_pool(name="mdcn", bufs=CT))
        ypool = ep(tc.tile_pool(name="ypool", bufs=2 * OT))
        outp = ep(tc.tile_pool(name="outp", bufs=2))
        statp = ep(tc.tile_pool(name="statp", bufs=1))
        ps_tp = ep(tc.tile_pool(name="ps_tp", bufs=2, space="PSUM"))
        ps_mm = ep(tc.tile_pool(name="ps_mm", bufs=2, space="PSUM"))

        # ---- constants ----
        ident_h = constp.tile([128, 128], F16)
        make_identity(nc, ident_h[:])
        ones_col = constp.tile([128, 1], F32)
        nc.vector.memset(ones_col[:], 1.0)

        # per-channel partial sums of y and y^2: col = (ot*2 + bi)*2 + h
        part_s1 = statp.tile([128, OT * B_LOC * NH], F32)
        part_s2 = statp.tile([128, OT * B_LOC * NH], F32)

        y_tiles = {}  # (bi, ot) -> tile (128, N) fp16

        # ---- phase 1 (both batches): load, norms, normalize, layouts ----
        xn_ct_b = {}
        xn_nc_b = {}
        for bi in range(B_LOC):
            x_ct = [
                xload.tile([128, N], F32, tag="x", name=f"x{bi}_{ct}")
                for ct in range(CT)
            ]
            xsq_ct = [
                sqp.tile([128, N], F32, tag="xsq", name=f"xsq{bi}_{ct}")
                for ct in range(CT)
            ]
            # chunk loads and squares (quarters for batch 0 to shorten the
            # startup chain, halves after) so the norm chain starts early
            nq = 4 if bi == 0 else NH
            qw = N // nq
            for h in range(nq):
                hs = slice(h * qw, (h + 1) * qw)
                for ct in range(CT):
                    nc.sync.dma_start(
                        out=x_ct[ct][:, hs],
                        in_=x_in[bi, ct * 128:(ct + 1) * 128, hs],
                    )
                    nc.scalar.activation(
                        xsq_ct[ct][:, hs], x_ct[ct][:, hs], AF.Square
                    )

            rnorm_row = rowp.tile([1, N], F32, tag="rnorm")
            for h in range(nq):
                hs = slice(h * qw, (h + 1) * qw)
                ps = ps_mm.tile([1, qw], F32, tag="cv", name=f"srow{bi}_{h}")
                for ct in range(CT):
                    nc.tensor.matmul(
                        out=ps[:],
                        lhsT=ones_col[:],
                        rhs=xsq_ct[ct][:, hs],
                        start=(ct == 0),
                        stop=(ct == CT - 1),
                    )
                # sqrt on ACT (PSUM -> SBUF), then reciprocal on DVE
                srt = rowp.tile([1, qw], F32, tag="srt", name=f"srt{bi}_{h}")
                nc.scalar.activation(srt[:], ps[:], AF.Sqrt)
                nc.vector.reciprocal(rnorm_row[:, hs], srt[:])

            # broadcast row across partitions (GpSimd custom op)
            rnorm_bc = bcp.tile([128, N], F32, tag="rnorm_bc")
            nc.gpsimd.partition_broadcast(rnorm_bc[:], rnorm_row[:])

            # normalize: xn = x * rnorm (column-wise), fp16 out
            xn_ct = []
            for ct in range(CT):
                t = xnp.tile([128, N], F16, tag="xn")
                nc.vector.tensor_tensor(
                    t[:], x_ct[ct][:], rnorm_bc[:], op=AluOpType.mult
                )
                xn_ct.append(t)
            xn_ct_b[bi] = xn_ct

            # (N, C) layout via DMA-transpose XBAR; gather rows to DRAM
            xn_nc = xnncp.tile([128, NB, C], F16, tag="xn_nc")
            for ct in range(CT):
                nc.sync.dma_start(
                    out=xn_nc[:, :, ct * 128:(ct + 1) * 128],
                    in_=xn_ct[ct][:],
                    transpose=True,
                )
            nc.sync.dma_start(
                out=xn_rows[bi][:, :].rearrange("(rb p) c -> p rb c", p=128),
                in_=xn_nc[:],
            )
            xn_nc_b[bi] = xn_nc

        # ---- replicated weights / BN params (needed late; after x loads
        # so batch 0's norm chain leads the DMA queue) ----
        nbig_h = constp.tile([128, 128], F16)
        nc.scalar.activation(nbig_h[:], ident_h[:], AF.Copy, scale=-60000.0)
        wev = []
        wod = []
        for ct in range(CT):
            t = wpool.tile([128, OUT], F16, tag="wev", name=f"wev{ct}")
            nc.sync.dma_start(out=t[:], in_=wev_in[ct * 128:(ct + 1) * 128, :])
            wev.append(t)
            t = wpool.tile([128, OUT], F16, tag="wod", name=f"wod{ct}")
            nc.sync.dma_start(out=t[:], in_=wod_in[ct * 128:(ct + 1) * 128, :])
            wod.append(t)
        gamma4 = constp.tile([128, OT], F32)
        nc.sync.dma_start(out=gamma4[:], in_=gamma_in[:, :])
        beta4 = constp.tile([128, OT], F32)
        nc.sync.dma_start(out=beta4[:], in_=beta_in[:, :])

        # ---- per batch: row blocks (software-pipelined) then conv ----
        for bi in range(B_LOC):
            xn_ct = xn_ct_b[bi]
            xn_nc = xn_nc_b[bi]

            if bi == 0:
                pending_conv = []

            md_cn = []
            for ct in range(CT):
                md_cn.append(
                    mdcnp.tile([128, N], F16, tag="md_cn", name=f"md_cn{bi}_{ct}")
                )

            nbr_t = {}
            def conv_piece(ot, h, bi=bi, xn_ct=xn_ct, md_cn=md_cn):
                ots = slice(ot * 128, (ot + 1) * 128)
                if h == 0:
                    yt = ypool.tile(
                        [128, N], F16, tag="y", name=f"y{bi}_{ot}"
                    )
                    y_tiles[(bi, ot)] = yt
                else:
                    yt = y_tiles[(bi, ot)]
                hs = slice(h * 512, (h + 1) * 512)
                ps = ps_mm.tile(
                    [128, 512], F32, tag="cv", name=f"cps{bi}_{ot}_{h}"
                )
                for ct in range(CT):
                    nc.tensor.matmul(
                        out=ps[:],
                        lhsT=wev[ct][:, ots],
                        rhs=xn_ct[ct][:, hs],
                        start=(ct == 0),
                        stop=False,
                    )
                for ct in range(CT):
                    nc.tensor.matmul(
                        out=ps[:],
                        lhsT=wod[ct][:, ots],
                        rhs=md_cn[ct][:, hs],
                        start=False,
                        stop=(ct == CT - 1),
                    )
                # move PSUM->SBUF on ACT with fused per-channel sum
                col = (ot * B_LOC + bi) * NH + h
                nc.scalar.activation(
                    yt[:, hs],
                    ps[:],
                    AF.Copy,
                    accum_out=part_s1[:, col:col + 1],
                )
                # sumsq with fused per-channel sum; the final batch's h1
                # pieces flush at the tail where ACT is the critical chain,
                # so run those squares on the then-idle DVE instead
                sq_scr = sqp.tile(
                    [128, 512], F16, tag="ysq", name=f"ysq{bi}_{ot}_{h}"
                )
                if bi == B_LOC - 1 and h == NH - 1:
                    nc.vector.scalar_tensor_tensor(
                        out=sq_scr[:],
                        in0=yt[:, hs],
                        scalar=1.0,
                        in1=yt[:, hs],
                        op0=AluOpType.mult,
                        op1=AluOpType.mult,
                        accum_out=part_s2[:, col:col + 1],
                    )
                else:
                    nc.scalar.activation(
                        sq_scr[:],
                        yt[:, hs],
                        AF.Square,
                        accum_out=part_s2[:, col:col + 1],
                    )
                if dbg and h == NH - 1:
                    nc.sync.dma_start(
                        out=y_dump[bi, ot * 128:(ot + 1) * 128, :], in_=yt[:]
                    )


            my_h0_pieces = [
                (lambda ot=ot, f=conv_piece: f(ot, 0)) for ot in range(OT)
            ]
            my_h1_pieces = [
                (lambda ot=ot, f=conv_piece: f(ot, 1)) for ot in range(OT)
            ]


            def stage_a(rb, bi=bi, xn_ct=xn_ct):
                rbs = slice(rb * 128, (rb + 1) * 128)
                # Gram row block straight into a 2-bank PSUM tile; the top-k
                # scans read PSUM directly (f32 scores, no SBUF copy).
                ps = ps_mm.tile([128, N], F32, tag="mm")
                for h in range(NH):
                    hs = slice(h * 512, (h + 1) * 512)
                    # matmul group per half: 2 channel tiles, plus (for the
                    # half containing the diagonal) a -60000*I accumulation
                    # for self-exclusion (frees a DVE op per row block)
                    group = [
                        (ps[:, hs], xn_ct[ct][:, rbs], xn_ct[ct][:, hs])
                        for ct in range(CT)
                    ]
                    if h == rb // (NB // NH):
                        group.append(
                            (
                                ps[:, rb * 128:(rb + 1) * 128],
                                nbig_h[:],
                                ident_h[:],
                            )
                        )
                    for k, (o, lt, r) in enumerate(group):
                        nc.tensor.matmul(
                            out=o,
                            lhsT=lt,
                            rhs=r,
                            start=(k == 0),
                            stop=(k == len(group) - 1),
                        )

                # scores move to SBUF f32 on ACT: releases the PSUM tile
                # early and avoids the PSUM-access penalty on each DVE scan
                sc = scp.tile([128, N], F32, tag="sc")
                nc.scalar.copy(sc[:], ps[:])

                # top-16 (largest score == nearest): 8 + 8 on DVE
                idx16 = idxp.tile([128, K_G], U32, tag="idx")
                m8 = idxp.tile([128, 8], F32, tag="m8")
                nc.vector.max(out=m8[:], in_=sc[:])
                nc.vector.max_index(
                    out=idx16[:, 0:8], in_max=m8[:], in_values=sc[:]
                )
                # first half of the gathers can start as soon as the first
                # 8 indices are known
                nbr = nbrp.tile([128, K_G, C], F16, tag="nbr")
                for s in range(8):
                    nc.gpsimd.indirect_dma_start(
                        out=nbr[:, s, :],
                        out_offset=None,
                        in_=xn_rows[bi][:],
                        in_offset=IndirectOffsetOnAxis(
                            ap=idx16[:, s:s + 1], axis=0
                        ),
                    )
                nc.vector.match_replace(
                    out=sc[:],
                    in_to_replace=m8[:],
                    in_values=sc[:],
                    imm_value=float(-BIG),
                )
                m8b = idxp.tile([128, 8], F32, tag="m8b")
                nc.vector.max(out=m8b[:], in_=sc[:])
                nc.vector.max_index(
                    out=idx16[:, 8:16], in_max=m8b[:], in_values=sc[:]
                )
                if dbg:
                    nc.sync.dma_start(out=idx_dump[bi, rb], in_=idx16[:])

                for s in range(8, K_G):
                    nc.gpsimd.indirect_dma_start(
                        out=nbr[:, s, :],
                        out_offset=None,
                        in_=xn_rows[bi][:],
                        in_offset=IndirectOffsetOnAxis(
                            ap=idx16[:, s:s + 1], axis=0
                        ),
                    )
                if dbg:
                    nc.sync.dma_start(out=nbr_dump[bi, rb], in_=nbr[:])
                nbr_t[rb] = nbr

            def stage_b(rb, bi=bi, xn_nc=xn_nc, md_cn=md_cn, nbr_t=nbr_t):
                rbs = slice(rb * 128, (rb + 1) * 128)
                nbr = nbr_t.pop(rb)
                # min/max over the 16 neighbors (TT trees on DVE, fp16 2x)
                tmax = treep.tile([128, K_G // 2, C], F16, tag="tmax")
                tmin = treep.tile([128, K_G // 2, C], F16, tag="tmin")
                nc.vector.tensor_tensor(
                    tmax[:], nbr[:, 0:8, :], nbr[:, 8:16, :], op=AluOpType.max
                )
                nc.vector.tensor_tensor(
                    tmin[:], nbr[:, 0:8, :], nbr[:, 8:16, :], op=AluOpType.min
                )
                w_ = 4
                while w_ >= 1:
                    nc.vector.tensor_tensor(
                        tmax[:, 0:w_, :],
                        tmax[:, 0:w_, :],
                        tmax[:, w_:2 * w_, :],
                        op=AluOpType.max,
                    )
                    nc.vector.tensor_tensor(
                        tmin[:, 0:w_, :],
                        tmin[:, 0:w_, :],
                        tmin[:, w_:2 * w_, :],
                        op=AluOpType.min,
                    )
                    w_ //= 2

                # md = max(xn - min, max - xn)
                md_nc = mdncp.tile([128, C], F16, tag="md_nc")
                d1 = mdncp.tile([128, C], F16, tag="d1")
                nc.vector.tensor_tensor(
                    d1[:], xn_nc[:, rb, :], tmin[:, 0, :], op=AluOpType.subtract
                )
                nc.vector.tensor_tensor(
                    md_nc[:], tmax[:, 0, :], xn_nc[:, rb, :],
                    op=AluOpType.subtract,
                )
                nc.vector.tensor_tensor(
                    md_nc[:], md_nc[:], d1[:], op=AluOpType.max
                )

                # transpose md block into (C, N) tiles (PE + ACT copy)
                for ct in range(CT):
                    ps = ps_tp.tile([128, 128], F16, tag="tp")
                    nc.tensor.transpose(
                        out=ps[:],
                        in_=md_nc[:, ct * 128:(ct + 1) * 128],
                        identity=ident_h[:],
                    )
                    nc.scalar.copy(md_cn[ct][:, rbs], ps[:])

            # 2-stage software pipeline: trees(rb-2) issue after scans(rb)
            # so the in-order DVE never stalls on the gather latency; ready
            # conv pieces (previous batch, then this batch's h0 half once
            # rbs 0-3 are emitted) slot in to spread PSUM/ACT load
            for rb in range(NB):
                stage_a(rb)
                if rb >= 3:
                    stage_b(rb - 3)
                if rb == 6:
                    pending_conv.extend(my_h0_pieces)
                for _ in range(2):
                    if pending_conv:
                        pending_conv.pop(0)()
            for rb in range(NB - 3, NB):
                stage_b(rb)
                if pending_conv:
                    pending_conv.pop(0)()
            pending_conv.extend(my_h1_pieces)

            if dbg:
                for ct in range(CT):
                    nc.sync.dma_start(
                        out=md_dump[bi, ct * 128:(ct + 1) * 128, :],
                        in_=md_cn[ct][:],
                    )

        while pending_conv:
            pending_conv.pop(0)()

        # ---- BN stats: reduce partials, all-reduce across cores ----
        stats_sb = statp.tile([128, 2 * OT], F32)
        nc.vector.tensor_reduce(
            stats_sb[:, 0:OT],
            part_s1[:].rearrange("p (o q) -> p o q", q=B_LOC * NH),
            axis=AX,
            op=AluOpType.add,
        )
        nc.vector.tensor_reduce(
            stats_sb[:, OT:2 * OT],
            part_s2[:].rearrange("p (o q) -> p o q", q=B_LOC * NH),
            axis=AX,
            op=AluOpType.add,
        )
        if dbg:
            nc.sync.dma_start(out=s_dump[:, 0:OT * B_LOC], in_=part_s1[:])
            nc.sync.dma_start(
                out=s_dump[:, OT * B_LOC:2 * OT * B_LOC], in_=part_s2[:]
            )
        nc.gpsimd.dma_start(out=stats_in[:, :], in_=stats_sb[:])
        if collective:
            nc.gpsimd.collective_compute(
                "AllReduce",
                AluOpType.add,
                replica_groups=[list(range(N_CORES))],
                ins=[stats_in.ap().opt()],
                outs=[stats_out.ap().opt()],
            )
        else:
            # sim-only stand-in: single-core timing proxy for the 4KB AR
            nc.gpsimd.dma_start(out=stats_out[:, :], in_=stats_in[:, :])
        stats_red = statp.tile([128, 2 * OT], F32)
        nc.gpsimd.dma_start(out=stats_red[:], in_=stats_out[:, :])

        # mean/var/affine (per channel; channel c = partition p, col ot)
        inv_cnt = 1.0 / float(B * N)
        mean4 = statp.tile([128, OT], F32)
        nc.vector.tensor_scalar_mul(mean4[:], stats_red[:, 0:OT], inv_cnt)
        var4 = statp.tile([128, OT], F32)
        # var = s2/cnt - mean^2
        nc.vector.tensor_scalar_mul(var4[:], stats_red[:, OT:2 * OT], inv_cnt)
        msq = statp.tile([128, OT], F32)
        nc.vector.tensor_tensor(msq[:], mean4[:], mean4[:], op=AluOpType.mult)
        nc.vector.tensor_tensor(var4[:], var4[:], msq[:], op=AluOpType.subtract)
        # rstd = 1/sqrt(var+eps)
        nc.vector.tensor_scalar_add(var4[:], var4[:], float(BN_EPS))
        std4 = statp.tile([128, OT], F32)
        nc.scalar.activation(std4[:], var4[:], AF.Sqrt)
        rstd4 = statp.tile([128, OT], F32)
        nc.vector.reciprocal(rstd4[:], std4[:])
        a4 = statp.tile([128, OT], F32)
        nc.vector.tensor_tensor(a4[:], gamma4[:], rstd4[:], op=AluOpType.mult)
        b4 = statp.tile([128, OT], F32)
        # b4 = beta - mean * a
        nc.vector.scalar_tensor_tensor(
            out=b4[:],
            in0=mean4[:],
            scalar=-1.0,
            in1=a4[:],
            op0=AluOpType.mult,
            op1=AluOpType.mult,
        )
        nc.vector.tensor_tensor(b4[:], b4[:], beta4[:], op=AluOpType.add)

        # ---- fused BN + exact gelu on ACT (f32 out), then store ----
        for bi in range(B_LOC):
            for ot in range(OT):
                yt = y_tiles[(bi, ot)]
                ot_f32 = outp.tile([128, N], F16, tag="of")
                nc.scalar.activation(
                    ot_f32[:],
                    yt[:],
                    AF.Gelu if use_gelu else AF.Copy,
                    bias=b4[:, ot:ot + 1] if use_gelu else 0.0,
                    scale=a4[:, ot:ot + 1],
                )
                nc.sync.dma_start(
                    out=out_dram[bi, ot * 128:(ot + 1) * 128, :], in_=ot_f32[:]
                )

    nc.compile()
    return nc


_NC_CACHE = None


def _get_nc():
    global _NC_CACHE
    if _NC_CACHE is None:
        _NC_CACHE = build_kernel()
    return _NC_CACHE


def _prep_shared(w, gamma, beta):
    w = np.asarray(w, np.float32)
    wev = np.ascontiguousarray(w[:, 0::2].T.astype(np.float16))  # (C, OUT)
    wod = np.ascontiguousarray(w[:, 1::2].T.astype(np.float16))
    gamma4 = np.ascontiguousarray(
        np.asarray(gamma, np.float32).reshape(OT, 128).T
    )
    beta4 = np.ascontiguousarray(np.asarray(beta, np.float32).reshape(OT, 128).T)
    return wev, wod, gamma4, beta4


def kernel(x, w, b, gamma, beta):
    x = np.ascontiguousarray(np.asarray(x, np.float32))
    assert x.shape == (B, C, N), x.shape
    wev, wod, gamma4, beta4 = _prep_shared(w, gamma, beta)
    # b cancels exactly in training-mode BN (see module docstring).
    nc = _get_nc()
    in_maps = [
        {
            "x": np.ascontiguousarray(x[c * B_LOC:(c + 1) * B_LOC]),
            "wev": wev,
            "wod": wod,
            "gamma4": gamma4,
            "beta4": beta4,
        }
        for c in range(N_CORES)
    ]
    res = run_bass_kernel_spmd(nc, in_maps, core_ids=list(range(N_CORES)))
    out = np.concatenate([res.results[c]["out"] for c in range(N_CORES)], axis=0)
    return out[..., None].astype(np.float32)
